# revision 1
# baseline (speedup 1.0000x reference)
"""Single-head causal attention (B=8, T=2048, C=1024, H=64) on 8 TRN2 NeuronCores.

Strategy (data-parallel over batch, one batch element per core):
  - Host transposes x[b] -> xT [C, T], casts matmul operands to bf16, and
    prepacks all weights/constants into one SBUF-layout buffer.  DMA order
    is by first use (weight chunks 0-3, first xt pair, rest), x tiles travel
    in pairs (SP descriptor-gen ~0.65us/instr limits feed rate, not
    bandwidth), and 8 dependency-free dummy matmuls ramp the PE DVFS clock
    to 2.4GHz during the DMA head so real work starts at full speed.
  - Device, per core, per 512-wide t-block tb:
      proj(tb):  qT,kT = ([Wq|Wk].T @ xT_tb) packed in one PE pass; vT = Wv.T @ xT_tb
      evac(tb):  PSUM -> SBUF bf16 casts (kT via 64->0 partition-shift DVE copy)
      trans(tb): v chunks rebuilt in natural [s, h] layout via PE transpose,
                 with ONE ones-column appended (vext, [128, 65]) so the PV
                 matmul also produces the softmax denominator l in row 64.
                 65 output partitions instead of 128 halve the PE array
                 energy of the PV pass (the chip power-throttles the PE when
                 sustained power is too high, so energy == time; GpSimd is
                 kept idle for the same reason).
      attn(tb), per causally-trimmed s-chunk, single-chunk ST tiles
      software-pipelined at lookahead 4 (ST0..ST3, PV0, ST4, PV1, ...) so
      the exp dependency is pre-satisfied when each PV reaches the PE queue:
          ST[s, t] = kT_chunk.T @ qT_block            (PSUM, 1 bank/chunk)
          PT = exp(SCALE * ST)                        (one ACT per chunk, bf16 out)
          diag chunks: PT *= 0/1 triangular mask      (4x DVE mode, off the
                                                       ST->exp critical chain)
          PV[:, t] += vext_chunk.T @ PT               (rows 0-63 = out.T, 64 = l)
      epilogue, per 256-col half (starts before the block's last PV):
          DVE copies pv[0:65] -> bf16 SBUF, DMA to DRAM.
  - Host computes out = (pv_rows / l_row).T — the final normalize is part of
    the unshard/gather step (1M flops vs 17 GFLOP on device).
All matmul accumulation is fp32 (PSUM); bf16 operands give ~4e-3 l2 rel err.
"""

import numpy as np
import ml_dtypes
from contextlib import ExitStack

import concourse.bass as bass
from concourse import bacc
import concourse.mybir as mybir
import concourse.tile as tile
from concourse.bass import ts
from concourse.bass_utils import run_bass_kernel_spmd


B, T, C, H = 8, 2048, 1024, 64
P = 128
W_BLK = 512
HB = W_BLK // 2         # epilogue half-block
N_TB = T // W_BLK       # 4 t-blocks
N_C = C // P            # 8 contraction chunks
N_S = T // P            # 16 s-chunks
N_J = W_BLK // P        # 4 diagonal chunks per t-block
SCALE = float(H) ** -0.5
NEG = -1e30
HL = H + 1              # PV output rows: 64 out dims + 1 denominator row

MM_DT = mybir.dt.bfloat16
NP_MM = ml_dtypes.bfloat16
F32 = mybir.dt.float32

# consts tile layout (bf16 columns): [wqk_c | wv_c] per c-chunk, then ident.
# Per-chunk interleave so one small leading DMA covers the first proj chunks.
CH_W = 3 * H            # 192 cols per c-chunk (128 wqk + 64 wv)
ID_OFF = N_C * CH_W     # 1536
TRI_OFF = ID_OFF + H    # 1600: 0/1 lower-triangular mask (1 if t >= s)
CONST_W = TRI_OFF + P   # 1728


def build_nc() -> bacc.Bacc:
    nc = bacc.Bacc("TRN2")
    consts_d = nc.dram_tensor("consts", [P, CONST_W], MM_DT, kind="ExternalInput")
    # host pre-tiles xT so each [128, 512] tile is one contiguous 128KB read
    xT_d = nc.dram_tensor("xT", [N_TB, N_C, P, W_BLK], MM_DT, kind="ExternalInput")
    # unnormalized out rows 0-63 + denominator row 64, per t-block halves
    out_d = nc.dram_tensor("out", [HL, T], MM_DT, kind="ExternalOutput")

    with tile.TileContext(nc) as tc, ExitStack() as ctx:
        const = ctx.enter_context(tc.tile_pool(name="const", bufs=1))

        consts = const.tile([P, CONST_W], MM_DT)

        def wqk_c(c):
            return consts[:, c * CH_W: c * CH_W + 2 * H]

        def wv_c(c):
            return consts[:, c * CH_W + 2 * H: (c + 1) * CH_W]

        ident = consts[0:H, ID_OFF: ID_OFF + H]
        tri01 = consts[:, TRI_OFF: TRI_OFF + P]

        xt = {}

        def load_xt_pair(tb, c):
            # one DMA per 2 c-chunks: descriptor-gen on the SP queue
            # (~0.65us/instr) limits the feed rate, not the transfer
            t_ = const.tile([P, 2, W_BLK], MM_DT, name=f"xt{c}_{tb}")
            nc.sync.dma_start(
                t_, xT_d[tb, c:c + 2].rearrange("o p t -> p o t"))
            xt[(c, tb)] = t_[:, 0, :]
            xt[(c + 1, tb)] = t_[:, 1, :]

        # DMA order = first-use order: weight chunks 0-3 first, then the
        # first xt pair, then the rest
        nc.sync.dma_start(consts[:, 0:4 * CH_W], consts_d[:, 0:4 * CH_W])
        load_xt_pair(0, 0)
        nc.sync.dma_start(consts[:, 4 * CH_W:CONST_W],
                          consts_d[:, 4 * CH_W:CONST_W])
        load_xt_pair(0, 2)
        load_xt_pair(0, 4)
        load_xt_pair(0, 6)
        for tb in range(1, N_TB):
            for c in range(0, N_C, 2):
                load_xt_pair(tb, c)

        qT_blk = [const.tile([H, W_BLK], MM_DT, name=f"qT{tb}") for tb in range(N_TB)]
        kT_blk = [const.tile([H, W_BLK], MM_DT, name=f"kT{tb}") for tb in range(N_TB)]
        vT_blk = [const.tile([H, W_BLK], MM_DT, name=f"vT{tb}") for tb in range(N_TB)]
        # vext[s] = [v_nat(s) | ones-column]: PV yields out.T rows + l row
        vext = const.tile([P, N_S, HL], MM_DT, name="vext")
        nc.vector.memset(vext[:, :, H:HL], 1.0)

        with tc.tile_pool(name="ps_qk", bufs=1, space="PSUM") as ps_qk, \
             tc.tile_pool(name="ps_v", bufs=1, space="PSUM") as ps_v, \
             tc.tile_pool(name="ps_st", bufs=5, space="PSUM") as ps_st, \
             tc.tile_pool(name="ps_pv", bufs=1, space="PSUM") as ps_pv, \
             tc.tile_pool(name="ptp", bufs=7) as pt_pool, \
             tc.tile_pool(name="outp", bufs=4) as out_pool:

            # warm up the PE p-state during the DMA head: ~8 dummy matmuls on
            # garbage SBUF keep the PE continuously busy so the DVFS ramp
            # reaches full clock before the first real projection matmul.
            # Results land in st-pool tiles that real STs later overwrite
            # (start=True); the pool rotation orders them safely.
            warm_sb = const.tile([P, W_BLK], MM_DT, name="warm_sb")
            nc.vector.memset(warm_sb, 1.0)
            for wi in range(2):
                wtile = ps_st.tile([P, W_BLK], F32, tag="st", name=f"warm{wi}")
                for _ in range(4):
                    nc.tensor.matmul(wtile, warm_sb[:, 0:P], warm_sb[:],
                                     start=True, stop=True,
                                     skip_group_check=True)

            for tb in range(N_TB):
                # ---- proj(tb) ----
                qk_ps = ps_qk.tile([P, W_BLK], F32, tag="qk", name=f"qk{tb}")
                v_ps = ps_v.tile([H, W_BLK], F32, tag="v", name=f"v{tb}")
                # interleaved per c-chunk: each arriving xt tile feeds two
                # matmuls immediately, so the DMA stream keeps ahead of the PE
                for c in range(N_C):
                    nc.tensor.matmul(qk_ps, wqk_c(c), xt[(c, tb)],
                                     start=(c == 0), stop=(c == N_C - 1))
                    nc.tensor.matmul(v_ps, wv_c(c), xt[(c, tb)],
                                     start=(c == 0), stop=(c == N_C - 1))
                # vT first: the transpose chain (tr -> vext -> diag PV) is
                # longer than the qT/kT -> ST one; kT still lands before the
                # diagonal STs (5th+ in the ST stream) need it
                nc.vector.tensor_copy(vT_blk[tb][:], v_ps[:, :])
                nc.vector.tensor_copy(qT_blk[tb][:], qk_ps[0:H, :])
                # partition shift 64->0 (64-lane DVE op, quadrant-aligned)
                nc.vector.tensor_copy(kT_blk[tb][:], qk_ps[H:P, :])

                # ---- v transposes for this block (shares the qk psum tag) ----
                for j in range(N_J):
                    s = tb * N_J + j
                    tr = ps_qk.tile([P, H], MM_DT, tag="qk", name=f"tr{s}")
                    nc.tensor.transpose(tr, vT_blk[tb][:, ts(j, P)], ident)
                    nc.vector.tensor_copy(vext[:, s, 0:H], tr)

                # ---- attn(tb) ----
                pv = ps_pv.tile([HL, W_BLK], F32, tag="pv", name=f"pv{tb}")
                n_full = tb * N_J
                # (s_chunk, col offset within t-block, width)
                chunks = [(s, 0, W_BLK) for s in range(n_full)]
                chunks += [(n_full + j, j * P, W_BLK - j * P) for j in range(N_J)]
                n_ch = len(chunks)

                def emit_epi(half):
                    t0 = half * HB
                    ot = out_pool.tile([HL, HB], MM_DT, tag=f"ot{half}",
                                       name=f"ot{tb}_{half}")
                    nc.vector.tensor_copy(ot, pv[:, t0:t0 + HB])
                    nc.sync.dma_start(
                        out_d[:, tb * W_BLK + t0: tb * W_BLK + t0 + HB], ot)

                # single-chunk ST tiles (1 PSUM bank each) with lookahead-3
                # emission: PV(i) reaches the PE queue head only after the
                # exp(i) result is already in SBUF, so the PE pipeline never
                # drains on the exp dependency.
                pt_t = [None] * n_ch

                def emit_st(ci):
                    s, off, w = chunks[ci]
                    st_t = ps_st.tile([P, w], F32, tag="st", name=f"st{tb}_{ci}")
                    nc.tensor.matmul(st_t,
                                     kT_blk[s // N_J][:, ts(s % N_J, P)],
                                     qT_blk[tb][:, off:W_BLK],
                                     start=True, stop=True)
                    pt = pt_pool.tile([P, w], MM_DT, tag="pt",
                                      name=f"pt{tb}_{ci}")
                    nc.scalar.activation(pt, st_t,
                                         mybir.ActivationFunctionType.Exp,
                                         scale=SCALE)
                    if s >= n_full:
                        # diagonal: multiplicative 0/1 causal mask on the
                        # bf16 exp output (all-SBUF packed -> 4x DVE mode),
                        # off the ST->exp critical chain; exp of unmasked
                        # scores is finite so no inf*0 hazard
                        nc.vector.tensor_tensor(pt[:, 0:P], pt[:, 0:P],
                                                tri01, mybir.AluOpType.mult)
                    pt_t[ci] = pt

                def emit_pv(ci):
                    s, off, w = chunks[ci]
                    nc.tensor.matmul(pv[:, off:W_BLK], vext[:, s, :],
                                     pt_t[ci],
                                     start=(ci == 0), stop=(ci == n_ch - 1))

                LA = 4
                for ci in range(min(LA, n_ch)):
                    emit_st(ci)
                for ci in range(n_ch):
                    if ci + LA < n_ch:
                        emit_st(ci + LA)
                    emit_pv(ci)
                    # cols [0:256] are final once diag chunk j=1's PV is in
                    if ci == n_ch - 3:
                        emit_epi(0)
                emit_epi(1)

    nc.compile()
    return nc


_NC_CACHE = None


def _get_nc():
    global _NC_CACHE
    if _NC_CACHE is None:
        _NC_CACHE = build_nc()
    return _NC_CACHE


def prepare_in_maps(x, Wk, Wq, Wv):
    wqk = np.concatenate([np.asarray(Wq), np.asarray(Wk)], axis=1).astype(NP_MM)
    wv = np.asarray(Wv).astype(NP_MM)
    consts = np.zeros((P, CONST_W), dtype=NP_MM)
    wqk3 = wqk.reshape(N_C, P, 2 * H)
    wv3 = wv.reshape(N_C, P, H)
    for c in range(N_C):
        consts[:, c * CH_W: c * CH_W + 2 * H] = wqk3[c]
        consts[:, c * CH_W + 2 * H: (c + 1) * CH_W] = wv3[c]
    consts[0:H, ID_OFF:ID_OFF + H] = np.eye(H, dtype=NP_MM)
    ii = np.arange(P)
    consts[:, TRI_OFF:TRI_OFF + P] = (ii[None, :] >= ii[:, None]).astype(NP_MM)
    in_maps = []
    for b in range(B):
        xTb = np.asarray(x[b]).T.astype(NP_MM)  # [C, T]
        xT = np.ascontiguousarray(
            xTb.reshape(N_C, P, N_TB, W_BLK).transpose(2, 0, 1, 3)
        )  # [N_TB, N_C, 128, 512], each tile contiguous
        in_maps.append({"xT": xT, "consts": consts})
    return in_maps


def run(x, Wk, Wq, Wv, trace=False):
    nc = _get_nc()
    in_maps = prepare_in_maps(x, Wk, Wq, Wv)
    res = run_bass_kernel_spmd(nc, in_maps, core_ids=list(range(B)), trace=trace)
    outs = []
    for r in res.results:
        o = np.asarray(r["out"], dtype=np.float32)  # [65, T]
        outs.append((o[0:H, :] / o[H:HL, :]).T)     # normalize + transpose
    return np.stack(outs), res


def kernel(x, Wk, Wq, Wv):
    out, _ = run(x, Wk, Wq, Wv, trace=False)
    return out



# revision 3
# speedup vs baseline: 1.2425x; 1.2425x over previous
"""Single-head causal attention (B=8, T=2048, C=1024, H=64) on 8 TRN2 NeuronCores.

Strategy (data-parallel over batch, one batch element per core):
  - Host transposes x[b] -> xT [C, T] p-major as c-pairs ([128, 2, 512]
    tiles, 2KB contiguous per partition), casts operands to bf16, and packs
    weights/constants into one SBUF-layout buffer.  All DMAs issue from the
    SP HWDGE only (a single ring measures faster and far more stable than
    splitting desc-gen across SP+ACT); order is by first use: weights for
    c0-3, first two x pairs, remaining weights, rest of x.  8 dependency-
    free dummy matmuls ramp the PE clock (HAM gate) during the DMA head.
  - proj(tb): ALL qk MMs first in c order (qk(c7) gates evac -> ST -> exp
    of the next attn phase), then the v MMs in the PE slack behind them.
  - evac(tb): two [64,512] casts (qT aligned, kT 64-shifted, both to
    partition rows 0:63) + one v cast; DVE kept light here because the
    evac chain sits on the phase-transition critical path.
  - trans(tb): v chunks transposed to natural [s, h] layout two at a time
    via regular MMs against a [64,64] identity into one [128, 2, 64] psum
    tile + ONE strided DVE copy into vext; a ones-column in vext col 64
    makes the PV matmul also produce the softmax denominator row (l).
  - attn(tb): s-chunks processed in PAIRS: two serial half-array STs
    (K=64, PE rows 0:63 only -- keeps PE power density at the baseline
    level so the chip never trips its 5/6-clock power throttle, which
    costs far more than the concurrency would win) land in one [128,1024]
    2-bank psum tile; ONE merged exp per pair ((N+352)/1.2 ns per
    ACTIVATE, so halving the instruction count saves ~2.4us of ACT) --
    except each phase's first pair, split in two so the exp stream
    restarts before the kT evac finishes.  Diag chunks get a 0/1
    triangular mask multiply on the bf16 exp output (DVE).  PV accumulates
    [65, w] (64 out dims + denominator) with K=128.  proj/evac/trans for
    tb+1 drips in AFTER each pair's PVs (Tile's scheduler is emission-
    priority based; attn keeps priority, proj fills PE slack).
  - epilogue per 256-col half: DVE cast -> DMA out; host normalizes
    out = (pv_rows / l_row).T during the gather (1M flops vs 17 GFLOP).
All matmul accumulation is fp32 (PSUM); bf16 operands give ~4e-3 l2 rel err.
"""

import numpy as np
import ml_dtypes
from contextlib import ExitStack

import concourse.bass as bass
from concourse import bacc
import concourse.mybir as mybir
import concourse.tile as tile
from concourse.bass import ts
from concourse.bass_utils import run_bass_kernel_spmd


B, T, C, H = 8, 2048, 1024, 64
P = 128
W_BLK = 512
HB = W_BLK // 2         # epilogue half-block
N_TB = T // W_BLK       # 4 t-blocks
N_C = C // P            # 8 contraction chunks
N_S = T // P            # 16 s-chunks
N_J = W_BLK // P        # 4 diagonal chunks per t-block
SCALE = float(H) ** -0.5
HL = H + 1              # PV output rows: 64 out dims + 1 denominator row

MM_DT = mybir.dt.bfloat16
NP_MM = ml_dtypes.bfloat16
F32 = mybir.dt.float32

# consts tile layout (bf16 columns): [wqk_c | wv_c] per c-chunk, then
# id2 = [I64; I64] stacked (transpose-and-merge of the col-tiled v psum
# halves via one regular matmul), then 0/1 lower-triangular mask.
CH_W = 3 * H            # 192 cols per c-chunk (128 wqk + 64 wv)
ID_OFF = N_C * CH_W     # 1536
TRI_OFF = ID_OFF + H    # 1600
CONST_W = TRI_OFF + P   # 1728

N_WARM = 8


def build_nc() -> bacc.Bacc:
    nc = bacc.Bacc("TRN2")
    consts_d = nc.dram_tensor("consts", [P, CONST_W], MM_DT, kind="ExternalInput")
    # host pre-tiles xT p-major as c-pairs: 2KB contiguous lines per
    # partition, 256KB sem granularity so proj consumes x as it trickles in
    xTp_d = nc.dram_tensor("xTp", [N_TB, 4, P, 2, W_BLK], MM_DT,
                           kind="ExternalInput")
    # unnormalized out rows 0-63 + denominator row 64, per t-block halves
    out_d = nc.dram_tensor("out", [HL, T], MM_DT, kind="ExternalOutput")

    with tile.TileContext(nc) as tc, ExitStack() as ctx:
        const = ctx.enter_context(tc.tile_pool(name="const", bufs=1))

        consts = const.tile([P, CONST_W], MM_DT)

        def wqk_c(c):
            return consts[:, c * CH_W: c * CH_W + 2 * H]

        def wv_c(c):
            return consts[:, c * CH_W + 2 * H: (c + 1) * CH_W]

        id2 = consts[0:H, ID_OFF: ID_OFF + H]
        tri01 = consts[:, TRI_OFF: TRI_OFF + P]

        xt = {}

        def load_pair(tb, cp, eng):
            t_ = const.tile([P, 2, W_BLK], MM_DT, name=f"xtp{tb}_{cp}")
            eng.dma_start(t_, xTp_d[tb, cp])
            xt[(2 * cp, tb)] = t_[:, 0, :]
            xt[(2 * cp + 1, tb)] = t_[:, 1, :]

        # DMA issue order: weights for c0-3 split across both HWDGE
        # sequencers (everything proj(0)'s first quad needs), then tb0 x,
        # then the rest of the weights, then tb1-3 x.  ACT-issued DMAs are
        # all in the head, before the first EXP hits its queue.
        W1 = 4 * CH_W
        nc.sync.dma_start(consts[:, 0:W1], consts_d[:, 0:W1])
        load_pair(0, 0, nc.sync)
        load_pair(0, 1, nc.sync)
        nc.sync.dma_start(consts[:, W1:CONST_W], consts_d[:, W1:CONST_W])
        load_pair(0, 2, nc.sync)
        load_pair(0, 3, nc.sync)
        for tb in range(1, N_TB):
            for cp in range(N_J):
                load_pair(tb, cp, nc.sync)

        # per-tb SBUF tiles: qk_A = [qT | kT] aligned, qk_B = [kT | qT]
        # partition-swapped, vv = v in [even-c sum | odd-c sum] halves
        qk_A = [const.tile([P, W_BLK], MM_DT, name=f"qkA{tb}") for tb in range(N_TB)]
        qk_B = [const.tile([P, W_BLK], MM_DT, name=f"qkB{tb}") for tb in range(N_TB)]
        vv = [const.tile([H, W_BLK], MM_DT, name=f"vv{tb}") for tb in range(N_TB)]
        # vext[s] = [v_nat(s) | ones-column]: PV yields out.T rows + l row
        vext = const.tile([P, N_S, HL], MM_DT, name="vext")
        nc.vector.memset(vext[:, :, H:HL], 1.0)

        with tc.tile_pool(name="ps_qk", bufs=1, space="PSUM") as ps_qk, \
             tc.tile_pool(name="ps_v", bufs=1, space="PSUM") as ps_v, \
             tc.tile_pool(name="ps_st", bufs=2, space="PSUM") as ps_st, \
             tc.tile_pool(name="ps_tr", bufs=1, space="PSUM") as ps_tr, \
             tc.tile_pool(name="ps_pv", bufs=1, space="PSUM") as ps_pv, \
             tc.tile_pool(name="ptp", bufs=4) as pt_pool, \
             tc.tile_pool(name="outp", bufs=4) as out_pool:

            # warm up the PE p-state during the DMA head so the HAM clock
            # gate is released before real work
            warm_sb = const.tile([P, W_BLK], MM_DT, name="warm_sb")
            nc.vector.memset(warm_sb, 1.0)
            for wi in range(2):
                wtile = ps_st.tile([P, 2 * W_BLK], F32, tag="st", name=f"warm{wi}")
                for _ in range(N_WARM // 2):
                    nc.tensor.matmul(wtile[:, 0:W_BLK], warm_sb[:, 0:P], warm_sb[:],
                                     start=True, stop=True,
                                     skip_group_check=True)

            # ---------- proj / evac / trans emitters ----------
            def emit_proj_qk(tb, c):
                nc.tensor.matmul(proj_ps[tb]["qk"], wqk_c(c), xt[(c, tb)],
                                 start=(c == 0), stop=(c == N_C - 1))

            def emit_proj_v(tb, c):
                nc.tensor.matmul(proj_ps[tb]["v"], wv_c(c), xt[(c, tb)],
                                 start=(c == 0), stop=(c == N_C - 1))

            def emit_evac(tb, which):
                # 2 casts: qT aligned and kT shifted, both to rows 0:64
                qk_ps = proj_ps[tb]["qk"]
                if which == 0:
                    nc.vector.tensor_copy(qk_A[tb][0:64, :], qk_ps[0:H, :])
                else:
                    nc.vector.tensor_copy(qk_B[tb][0:64, :], qk_ps[H:P, :])

            def emit_evac_v(tb):
                nc.vector.tensor_copy(vv[tb][:], proj_ps[tb]["v"][:, :])

            def emit_trans(tb, j):
                # transpose two chunks via regular MMs (tr = vv_chunk.T @ I)
                # into one psum tile, then ONE strided DVE copy for both
                s = tb * N_J + j
                tr2 = ps_tr.tile([P, 2, H], F32, tag="tr", name=f"tr{s}")
                nc.tensor.matmul(tr2[:, 0, :], vv[tb][:, ts(j, P)], id2,
                                 start=True, stop=True)
                nc.tensor.matmul(tr2[:, 1, :], vv[tb][:, ts(j + 1, P)], id2,
                                 start=True, stop=True)
                nc.vector.tensor_copy(vext[:, s:s + 2, 0:H], tr2)

            proj_ps = {}

            def alloc_proj_ps(tb):
                proj_ps[tb] = {
                    "qk": ps_qk.tile([P, W_BLK], F32, tag="qk", name=f"qk{tb}"),
                    "v": ps_v.tile([H, W_BLK], F32, tag="v", name=f"v{tb}"),
                }

            def proj_work(tb):
                """Closures for proj+evac+trans of tb, emitted in order."""
                items = [lambda tb=tb: alloc_proj_ps(tb)]
                # ALL qk first (qk(c7) gates evac -> ST -> exp of the next
                # attn phase); v MMs follow in the PE slack behind the evacs
                for c in range(N_C):
                    items.append(lambda tb=tb, c=c: emit_proj_qk(tb, c))
                for c in range(N_C):
                    items.append(lambda tb=tb, c=c: emit_proj_v(tb, c))
                for w in range(2):
                    items.append(lambda tb=tb, w=w: emit_evac(tb, w))
                items.append(lambda tb=tb: emit_evac_v(tb))
                for j in range(0, N_J, 2):
                    items.append(lambda tb=tb, j=j: emit_trans(tb, j))
                return items

            # ---------- attn ----------
            def emit_attn(tb, filler):
                """attn(tb); filler items drip-fed between pair slots."""
                n_full = tb * N_J
                # pair list: (s_lo, off_lo, w_lo, s_hi, off_hi, w_hi)
                pairs = []
                for i in range(0, n_full, 2):
                    pairs.append((i, 0, W_BLK, i + 1, 0, W_BLK))
                pairs.append((n_full, 0, W_BLK,
                              n_full + 1, P, W_BLK - P))
                pairs.append((n_full + 2, 2 * P, W_BLK - 2 * P,
                              n_full + 3, 3 * P, W_BLK - 3 * P))
                n_p = len(pairs)
                F = n_p - 2  # index of first diag pair

                pv = ps_pv.tile([HL, W_BLK], F32, tag="pv", name=f"pv{tb}")
                pt_t = [None] * n_p

                def kq(s, off, w):
                    tbk, j = s // N_J, s % N_J
                    return (qk_B[tbk][0:64, ts(j, P)],
                            qk_A[tb][0:64, off:W_BLK])

                def emit_st_pair(p):
                    s0, o0, w0, s1, o1, w1 = pairs[p]
                    st2 = ps_st.tile([P, 2 * W_BLK], F32, tag="st",
                                     name=f"st{tb}_{p}")
                    pt = pt_pool.tile([P, 2 * W_BLK], MM_DT, tag="pt",
                                      name=f"pt{tb}_{p}")
                    k0, q0 = kq(s0, o0, w0)
                    k1, q1 = kq(s1, o1, w1)
                    # two serial half-array STs (K=64, rows 0:63 only --
                    # keeps PE power density at the baseline level so the
                    # chip never trips the P0 5/6-clock throttle) into one
                    # 2-bank tile: even -> bank A [0:w0], odd -> bank B
                    # [512:512+w1]; ONE exp per pair halves the ACT
                    # instruction overhead ((N+352)/1.2 ns per ACTIVATE).
                    nc.tensor.matmul(st2[:, 0:w0], k0, q0,
                                     start=True, stop=True,
                                     skip_group_check=True)
                    nc.tensor.matmul(st2[:, W_BLK:W_BLK + w1], k1, q1,
                                     start=True, stop=True,
                                     skip_group_check=True)
                    if p != 0 and p <= F:
                        # one exp over both banks (contiguous [0 : 512+w1])
                        nc.scalar.activation(pt[:, 0:W_BLK + w1],
                                             st2[:, 0:W_BLK + w1],
                                             mybir.ActivationFunctionType.Exp,
                                             scale=SCALE)
                    else:
                        # first pair of the phase (fire exp-A as soon as the
                        # even chunk's ST lands, before the hi-half evac
                        # casts finish) and the corner pair: two exps
                        nc.scalar.activation(pt[:, 0:w0], st2[:, 0:w0],
                                             mybir.ActivationFunctionType.Exp,
                                             scale=SCALE)
                        nc.scalar.activation(pt[:, W_BLK:W_BLK + w1],
                                             st2[:, W_BLK:W_BLK + w1],
                                             mybir.ActivationFunctionType.Exp,
                                             scale=SCALE)
                    if p >= F:
                        # diag masks on the bf16 exp output, off the critical
                        # ST->exp chain
                        nc.vector.tensor_tensor(pt[:, 0:P], pt[:, 0:P],
                                                tri01, mybir.AluOpType.mult)
                        nc.vector.tensor_tensor(
                            pt[:, W_BLK:W_BLK + P], pt[:, W_BLK:W_BLK + P],
                            tri01, mybir.AluOpType.mult)
                    pt_t[p] = pt

                def emit_pv_pair(p):
                    s0, o0, w0, s1, o1, w1 = pairs[p]
                    pt = pt_t[p]
                    nc.tensor.matmul(pv[:, o0:W_BLK], vext[:, s0, :],
                                     pt[:, 0:w0],
                                     start=(p == 0), stop=False)
                    nc.tensor.matmul(pv[:, o1:W_BLK], vext[:, s1, :],
                                     pt[:, W_BLK:W_BLK + w1],
                                     start=False, stop=(p == n_p - 1))

                def emit_epi(half):
                    t0 = half * HB
                    ot = out_pool.tile([HL, HB], MM_DT, tag=f"ot{half}",
                                       name=f"ot{tb}_{half}")
                    nc.vector.tensor_copy(ot, pv[:, t0:t0 + HB])
                    nc.sync.dma_start(
                        out_d[:, tb * W_BLK + t0: tb * W_BLK + t0 + HB], ot)

                fi = 0  # filler cursor

                def drip(k):
                    nonlocal fi
                    for _ in range(k):
                        if fi < len(filler):
                            filler[fi]()
                            fi += 1

                LA = 2
                for p in range(min(LA, n_p)):
                    emit_st_pair(p)
                # spread filler evenly, emitted AFTER each slot's PVs so
                # attn STs/PVs keep scheduler priority over next-tb proj
                per_slot = (len(filler) + n_p - 1) // n_p if n_p else 0
                for p in range(n_p):
                    if p + LA < n_p:
                        emit_st_pair(p + LA)
                    emit_pv_pair(p)
                    drip(per_slot)
                    if p == F:
                        emit_epi(0)
                emit_epi(1)
                drip(len(filler))  # leftovers

            # ---------- main schedule ----------
            # proj(tb+1) is drip-fed into attn(tb) so its MMs get PE slots
            # (the Tile scheduler is dep+priority driven; earlier emission =
            # higher priority) and the evac casts finish before attn(tb)
            # drains -- each attn phase then starts without an ACT gap.
            for it in proj_work(0):
                it()
            for tb in range(N_TB):
                filler = proj_work(tb + 1) if tb + 1 < N_TB else []
                emit_attn(tb, filler)

    nc.compile()
    return nc


_NC_CACHE = None


def _get_nc():
    global _NC_CACHE
    if _NC_CACHE is None:
        _NC_CACHE = build_nc()
    return _NC_CACHE


def prepare_in_maps(x, Wk, Wq, Wv):
    wqk = np.concatenate([np.asarray(Wq), np.asarray(Wk)], axis=1).astype(NP_MM)
    wv = np.asarray(Wv).astype(NP_MM)
    consts = np.zeros((P, CONST_W), dtype=NP_MM)
    wqk3 = wqk.reshape(N_C, P, 2 * H)
    wv3 = wv.reshape(N_C, P, H)
    for c in range(N_C):
        consts[:, c * CH_W: c * CH_W + 2 * H] = wqk3[c]
        consts[:, c * CH_W + 2 * H: (c + 1) * CH_W] = wv3[c]
    consts[0:H, ID_OFF:ID_OFF + H] = np.eye(H, dtype=NP_MM)
    ii = np.arange(P)
    consts[:, TRI_OFF:TRI_OFF + P] = (ii[None, :] >= ii[:, None]).astype(NP_MM)
    in_maps = []
    for b in range(B):
        xTb = np.asarray(x[b]).T.astype(NP_MM)  # [C, T]
        arr = xTb.reshape(N_C, P, N_TB, W_BLK).transpose(2, 0, 1, 3)  # [tb,c,p,t]
        xTp = np.ascontiguousarray(
            arr.reshape(N_TB, 4, 2, P, W_BLK).transpose(0, 1, 3, 2, 4))
        in_maps.append({"xTp": xTp, "consts": consts})
    return in_maps


def run(x, Wk, Wq, Wv, trace=False):
    nc = _get_nc()
    in_maps = prepare_in_maps(x, Wk, Wq, Wv)
    res = run_bass_kernel_spmd(nc, in_maps, core_ids=list(range(B)), trace=trace)
    outs = []
    for r in res.results:
        o = np.asarray(r["out"], dtype=np.float32)  # [65, T]
        outs.append((o[0:H, :] / o[H:HL, :]).T)     # normalize + transpose
    return np.stack(outs), res


def kernel(x, Wk, Wq, Wv):
    out, _ = run(x, Wk, Wq, Wv, trace=False)
    return out


# revision 4
# speedup vs baseline: 1.2922x; 1.0400x over previous
"""Single-head causal attention (B=8, T=2048, C=1024, H=64) on 8 TRN2 NeuronCores.

Strategy (data-parallel over batch, one batch element per core):
  - Host transposes x[b] -> xT [C, T] p-major as c-pairs ([128, 2, 512]
    tiles, 2KB contiguous per partition), casts operands to bf16, and packs
    weights/constants into one SBUF-layout buffer.  All DMAs issue from the
    SP HWDGE only (a single ring measures faster and far more stable than
    splitting desc-gen across SP+ACT); order is by first use: weights for
    c0-3, first two x pairs, remaining weights, rest of x.  8 dependency-
    free dummy matmuls ramp the PE clock (HAM gate) during the DMA head.
  - proj(tb): ALL qk MMs first in c order (qk(c7) gates evac -> ST -> exp
    of the next attn phase), then the v MMs in the PE slack behind them.
  - evac(tb): two [64,512] casts (qT aligned, kT 64-shifted, both to
    partition rows 0:63) + one v cast; DVE kept light here because the
    evac chain sits on the phase-transition critical path.
  - trans(tb): v chunks transposed to natural [s, h] layout two at a time
    via regular MMs against a [64,64] identity into one [128, 2, 64] psum
    tile + ONE strided DVE copy into vext; a ones-column in vext col 64
    makes the PV matmul also produce the softmax denominator row (l).
  - attn(tb): s-chunks processed in PAIRS into one [128,1024] 2-bank psum
    tile.  tb 0-2: two serial half-array STs (K=64, PE rows 0:63 only --
    keeps sustained PE power density low so the chip doesn't trip its
    5/6-clock power throttle, which costs far more than ST concurrency
    wins).  attn(3), which is otherwise PE-bound, uses a 2x2 PE-array
    grid (even chunk on K-rows 0:63, odd on 64:127, 128 outputs split
    into col-halves -> 4 MMs on disjoint row/col groups run concurrently)
    -- a bounded ~9us full-array window that stays under the power
    threshold.  ONE merged exp per pair ((N+352)/1.2 ns per ACTIVATE, so
    halving the instruction count saves ~2.4us of ACT) --
    except each phase's first pair, split in two so the exp stream
    restarts before the kT evac finishes.  Diag chunks get a 0/1
    triangular mask multiply on the bf16 exp output (DVE).  PV accumulates
    [65, w] (64 out dims + denominator) with K=128.  proj/evac/trans for
    tb+1 drips in AFTER each pair's PVs (Tile's scheduler is emission-
    priority based; attn keeps priority, proj fills PE slack).
  - epilogue per 256-col half: DVE cast -> DMA out; host normalizes
    out = (pv_rows / l_row).T during the gather (1M flops vs 17 GFLOP).
All matmul accumulation is fp32 (PSUM); bf16 operands give ~4e-3 l2 rel err.
"""

import numpy as np
import ml_dtypes
from contextlib import ExitStack

import concourse.bass as bass
from concourse import bacc
import concourse.mybir as mybir
import concourse.tile as tile
from concourse.bass import ts
from concourse.bass_utils import run_bass_kernel_spmd


B, T, C, H = 8, 2048, 1024, 64
P = 128
W_BLK = 512
HB = W_BLK // 2         # epilogue half-block
N_TB = T // W_BLK       # 4 t-blocks
N_C = C // P            # 8 contraction chunks
N_S = T // P            # 16 s-chunks
N_J = W_BLK // P        # 4 diagonal chunks per t-block
SCALE = float(H) ** -0.5
HL = H + 1              # PV output rows: 64 out dims + 1 denominator row

MM_DT = mybir.dt.bfloat16
NP_MM = ml_dtypes.bfloat16
F32 = mybir.dt.float32

# consts tile layout (bf16 columns): [wqk_c | wv_c] per c-chunk, then
# id2 = [I64; I64] stacked (transpose-and-merge of the col-tiled v psum
# halves via one regular matmul), then 0/1 lower-triangular mask.
CH_W = 3 * H            # 192 cols per c-chunk (128 wqk + 64 wv)
ID_OFF = N_C * CH_W     # 1536
TRI_OFF = ID_OFF + H    # 1600
CONST_W = TRI_OFF + P   # 1728

N_WARM = 8


def build_nc() -> bacc.Bacc:
    nc = bacc.Bacc("TRN2")
    consts_d = nc.dram_tensor("consts", [P, CONST_W], MM_DT, kind="ExternalInput")
    # host pre-tiles xT p-major as c-pairs: 2KB contiguous lines per
    # partition, 256KB sem granularity so proj consumes x as it trickles in
    xTp_d = nc.dram_tensor("xTp", [N_TB, 4, P, 2, W_BLK], MM_DT,
                           kind="ExternalInput")
    # unnormalized out rows 0-63 + denominator row 64, per t-block halves
    out_d = nc.dram_tensor("out", [HL, T], MM_DT, kind="ExternalOutput")

    with tile.TileContext(nc) as tc, ExitStack() as ctx:
        const = ctx.enter_context(tc.tile_pool(name="const", bufs=1))

        consts = const.tile([P, CONST_W], MM_DT)

        def wqk_c(c):
            return consts[:, c * CH_W: c * CH_W + 2 * H]

        def wv_c(c):
            return consts[:, c * CH_W + 2 * H: (c + 1) * CH_W]

        id2 = consts[0:H, ID_OFF: ID_OFF + H]
        tri01 = consts[:, TRI_OFF: TRI_OFF + P]

        xt = {}

        def load_pair(tb, cp, eng):
            t_ = const.tile([P, 2, W_BLK], MM_DT, name=f"xtp{tb}_{cp}")
            eng.dma_start(t_, xTp_d[tb, cp])
            xt[(2 * cp, tb)] = t_[:, 0, :]
            xt[(2 * cp + 1, tb)] = t_[:, 1, :]

        # DMA issue order: weights for c0-3 split across both HWDGE
        # sequencers (everything proj(0)'s first quad needs), then tb0 x,
        # then the rest of the weights, then tb1-3 x.  ACT-issued DMAs are
        # all in the head, before the first EXP hits its queue.
        W1 = 4 * CH_W
        nc.sync.dma_start(consts[:, 0:W1], consts_d[:, 0:W1])
        load_pair(0, 0, nc.sync)
        load_pair(0, 1, nc.sync)
        nc.sync.dma_start(consts[:, W1:CONST_W], consts_d[:, W1:CONST_W])
        load_pair(0, 2, nc.sync)
        load_pair(0, 3, nc.sync)
        for tb in range(1, N_TB):
            for cp in range(N_J):
                load_pair(tb, cp, nc.sync)

        # per-tb SBUF tiles: qk_A = [qT | kT] aligned, qk_B = [kT | qT]
        # partition-swapped, vv = v in [even-c sum | odd-c sum] halves
        qk_A = [const.tile([P, W_BLK], MM_DT, name=f"qkA{tb}") for tb in range(N_TB)]
        qk_B = [const.tile([P, W_BLK], MM_DT, name=f"qkB{tb}") for tb in range(N_TB)]
        vv = [const.tile([H, W_BLK], MM_DT, name=f"vv{tb}") for tb in range(N_TB)]
        # vext[s] = [v_nat(s) | ones-column]: PV yields out.T rows + l row
        vext = const.tile([P, N_S, HL], MM_DT, name="vext")
        nc.vector.memset(vext[:, :, H:HL], 1.0)

        with tc.tile_pool(name="ps_qk", bufs=1, space="PSUM") as ps_qk, \
             tc.tile_pool(name="ps_v", bufs=1, space="PSUM") as ps_v, \
             tc.tile_pool(name="ps_st", bufs=2, space="PSUM") as ps_st, \
             tc.tile_pool(name="ps_tr", bufs=1, space="PSUM") as ps_tr, \
             tc.tile_pool(name="ps_pv", bufs=1, space="PSUM") as ps_pv, \
             tc.tile_pool(name="ptp", bufs=4) as pt_pool, \
             tc.tile_pool(name="outp", bufs=4) as out_pool:

            # warm up the PE p-state during the DMA head so the HAM clock
            # gate is released before real work
            warm_sb = const.tile([P, W_BLK], MM_DT, name="warm_sb")
            nc.vector.memset(warm_sb, 1.0)
            for wi in range(2):
                wtile = ps_st.tile([P, 2 * W_BLK], F32, tag="st", name=f"warm{wi}")
                for _ in range(N_WARM // 2):
                    nc.tensor.matmul(wtile[:, 0:W_BLK], warm_sb[:, 0:P], warm_sb[:],
                                     start=True, stop=True,
                                     skip_group_check=True)

            # ---------- proj / evac / trans emitters ----------
            def emit_proj_qk(tb, c):
                nc.tensor.matmul(proj_ps[tb]["qk"], wqk_c(c), xt[(c, tb)],
                                 start=(c == 0), stop=(c == N_C - 1))

            def emit_proj_v(tb, c):
                nc.tensor.matmul(proj_ps[tb]["v"], wv_c(c), xt[(c, tb)],
                                 start=(c == 0), stop=(c == N_C - 1))

            def emit_evac(tb, which):
                # A: aligned full-height (qT rows 0:64 + kT rows 64:128),
                # B1: kT shifted to rows 0:64; tb3 also gets B2 (qT shifted
                # to rows 64:128) for its row-split grid STs
                qk_ps = proj_ps[tb]["qk"]
                if which == 0:
                    nc.vector.tensor_copy(qk_A[tb][:, :], qk_ps[:, :])
                elif which == 1:
                    nc.vector.tensor_copy(qk_B[tb][0:64, :], qk_ps[H:P, :])
                else:
                    nc.vector.tensor_copy(qk_B[tb][64:128, :], qk_ps[0:H, :])

            def emit_evac_v(tb):
                nc.vector.tensor_copy(vv[tb][:], proj_ps[tb]["v"][:, :])

            def emit_trans(tb, j):
                # transpose two chunks via regular MMs (tr = vv_chunk.T @ I)
                # into one psum tile, then ONE strided DVE copy for both
                s = tb * N_J + j
                tr2 = ps_tr.tile([P, 2, H], F32, tag="tr", name=f"tr{s}")
                nc.tensor.matmul(tr2[:, 0, :], vv[tb][:, ts(j, P)], id2,
                                 start=True, stop=True)
                nc.tensor.matmul(tr2[:, 1, :], vv[tb][:, ts(j + 1, P)], id2,
                                 start=True, stop=True)
                nc.vector.tensor_copy(vext[:, s:s + 2, 0:H], tr2)

            proj_ps = {}

            def alloc_proj_ps(tb):
                proj_ps[tb] = {
                    "qk": ps_qk.tile([P, W_BLK], F32, tag="qk", name=f"qk{tb}"),
                    "v": ps_v.tile([H, W_BLK], F32, tag="v", name=f"v{tb}"),
                }

            def proj_work(tb):
                """Closures for proj+evac+trans of tb, emitted in order."""
                items = [lambda tb=tb: alloc_proj_ps(tb)]
                # ALL qk first (qk(c7) gates evac -> ST -> exp of the next
                # attn phase); v MMs follow in the PE slack behind the evacs
                for c in range(N_C):
                    items.append(lambda tb=tb, c=c: emit_proj_qk(tb, c))
                for c in range(N_C):
                    items.append(lambda tb=tb, c=c: emit_proj_v(tb, c))
                n_ev = 3 if tb == N_TB - 1 else 2
                for w in range(n_ev):
                    items.append(lambda tb=tb, w=w: emit_evac(tb, w))
                items.append(lambda tb=tb: emit_evac_v(tb))
                for j in range(0, N_J, 2):
                    items.append(lambda tb=tb, j=j: emit_trans(tb, j))
                return items

            # ---------- attn ----------
            def emit_attn(tb, filler):
                """attn(tb); filler items drip-fed between pair slots."""
                n_full = tb * N_J
                # pair list: (s_lo, off_lo, w_lo, s_hi, off_hi, w_hi)
                pairs = []
                for i in range(0, n_full, 2):
                    pairs.append((i, 0, W_BLK, i + 1, 0, W_BLK))
                pairs.append((n_full, 0, W_BLK,
                              n_full + 1, P, W_BLK - P))
                pairs.append((n_full + 2, 2 * P, W_BLK - 2 * P,
                              n_full + 3, 3 * P, W_BLK - 3 * P))
                n_p = len(pairs)
                F = n_p - 2  # index of first diag pair

                pv = ps_pv.tile([HL, W_BLK], F32, tag="pv", name=f"pv{tb}")
                pt_t = [None] * n_p

                def kq(s, off, w):
                    tbk, j = s // N_J, s % N_J
                    if tb == N_TB - 1 and s % 2 == 1:
                        return (qk_A[tbk][64:128, ts(j, P)],
                                qk_B[tb][64:128, off:W_BLK])
                    return (qk_B[tbk][0:64, ts(j, P)],
                            qk_A[tb][0:64, off:W_BLK])

                def emit_st_pair(p):
                    s0, o0, w0, s1, o1, w1 = pairs[p]
                    st2 = ps_st.tile([P, 2 * W_BLK], F32, tag="st",
                                     name=f"st{tb}_{p}")
                    pt = pt_pool.tile([P, 2 * W_BLK], MM_DT, tag="pt",
                                      name=f"pt{tb}_{p}")
                    k0, q0 = kq(s0, o0, w0)
                    k1, q1 = kq(s1, o1, w1)
                    # serial half-array STs (power-safe) for tb 0-2; the
                    # PE-bound attn(3) uses the 2x2 grid (even chunk K-rows
                    # 0:63, odd 64:127, col-halves -> 4 concurrent MMs)
                    if tb == N_TB - 1:
                        nc.tensor.matmul(st2[0:64, 0:w0], k0[:, 0:64], q0,
                                         start=True, stop=True,
                                         skip_group_check=True)
                        nc.tensor.matmul(st2[64:128, 0:w0], k0[:, 64:128],
                                         q0, start=True, stop=True,
                                         skip_group_check=True)
                        nc.tensor.matmul(st2[0:64, W_BLK:W_BLK + w1],
                                         k1[:, 0:64], q1,
                                         start=True, stop=True,
                                         skip_group_check=True)
                        nc.tensor.matmul(st2[64:128, W_BLK:W_BLK + w1],
                                         k1[:, 64:128], q1,
                                         start=True, stop=True,
                                         skip_group_check=True)
                    else:
                        nc.tensor.matmul(st2[:, 0:w0], k0, q0,
                                         start=True, stop=True,
                                         skip_group_check=True)
                        nc.tensor.matmul(st2[:, W_BLK:W_BLK + w1], k1, q1,
                                         start=True, stop=True,
                                         skip_group_check=True)
                    if p != 0 and p <= F:
                        # one exp over both banks (contiguous [0 : 512+w1])
                        nc.scalar.activation(pt[:, 0:W_BLK + w1],
                                             st2[:, 0:W_BLK + w1],
                                             mybir.ActivationFunctionType.Exp,
                                             scale=SCALE)
                    else:
                        # first pair of the phase (fire exp-A as soon as the
                        # even chunk's ST lands, before the hi-half evac
                        # casts finish) and the corner pair: two exps
                        nc.scalar.activation(pt[:, 0:w0], st2[:, 0:w0],
                                             mybir.ActivationFunctionType.Exp,
                                             scale=SCALE)
                        nc.scalar.activation(pt[:, W_BLK:W_BLK + w1],
                                             st2[:, W_BLK:W_BLK + w1],
                                             mybir.ActivationFunctionType.Exp,
                                             scale=SCALE)
                    if p >= F:
                        # diag masks on the bf16 exp output, off the critical
                        # ST->exp chain
                        nc.vector.tensor_tensor(pt[:, 0:P], pt[:, 0:P],
                                                tri01, mybir.AluOpType.mult)
                        nc.vector.tensor_tensor(
                            pt[:, W_BLK:W_BLK + P], pt[:, W_BLK:W_BLK + P],
                            tri01, mybir.AluOpType.mult)
                    pt_t[p] = pt

                def emit_pv_pair(p):
                    s0, o0, w0, s1, o1, w1 = pairs[p]
                    pt = pt_t[p]
                    nc.tensor.matmul(pv[:, o0:W_BLK], vext[:, s0, :],
                                     pt[:, 0:w0],
                                     start=(p == 0), stop=False)
                    nc.tensor.matmul(pv[:, o1:W_BLK], vext[:, s1, :],
                                     pt[:, W_BLK:W_BLK + w1],
                                     start=False, stop=(p == n_p - 1))

                def emit_epi(half):
                    t0 = half * HB
                    ot = out_pool.tile([HL, HB], MM_DT, tag=f"ot{half}",
                                       name=f"ot{tb}_{half}")
                    nc.vector.tensor_copy(ot, pv[:, t0:t0 + HB])
                    nc.sync.dma_start(
                        out_d[:, tb * W_BLK + t0: tb * W_BLK + t0 + HB], ot)

                fi = 0  # filler cursor

                def drip(k):
                    nonlocal fi
                    for _ in range(k):
                        if fi < len(filler):
                            filler[fi]()
                            fi += 1

                LA = 2
                for p in range(min(LA, n_p)):
                    emit_st_pair(p)
                # spread filler evenly, emitted AFTER each slot's PVs so
                # attn STs/PVs keep scheduler priority over next-tb proj
                per_slot = (len(filler) + n_p - 1) // n_p if n_p else 0
                for p in range(n_p):
                    if p + LA < n_p:
                        emit_st_pair(p + LA)
                    emit_pv_pair(p)
                    drip(per_slot)
                    if p == F:
                        emit_epi(0)
                emit_epi(1)
                drip(len(filler))  # leftovers

            # ---------- main schedule ----------
            # proj(tb+1) is drip-fed into attn(tb) so its MMs get PE slots
            # (the Tile scheduler is dep+priority driven; earlier emission =
            # higher priority) and the evac casts finish before attn(tb)
            # drains -- each attn phase then starts without an ACT gap.
            for it in proj_work(0):
                it()
            for tb in range(N_TB):
                filler = proj_work(tb + 1) if tb + 1 < N_TB else []
                emit_attn(tb, filler)

    nc.compile()
    return nc


_NC_CACHE = None


def _get_nc():
    global _NC_CACHE
    if _NC_CACHE is None:
        _NC_CACHE = build_nc()
    return _NC_CACHE


def prepare_in_maps(x, Wk, Wq, Wv):
    wqk = np.concatenate([np.asarray(Wq), np.asarray(Wk)], axis=1).astype(NP_MM)
    wv = np.asarray(Wv).astype(NP_MM)
    consts = np.zeros((P, CONST_W), dtype=NP_MM)
    wqk3 = wqk.reshape(N_C, P, 2 * H)
    wv3 = wv.reshape(N_C, P, H)
    for c in range(N_C):
        consts[:, c * CH_W: c * CH_W + 2 * H] = wqk3[c]
        consts[:, c * CH_W + 2 * H: (c + 1) * CH_W] = wv3[c]
    consts[0:H, ID_OFF:ID_OFF + H] = np.eye(H, dtype=NP_MM)
    ii = np.arange(P)
    consts[:, TRI_OFF:TRI_OFF + P] = (ii[None, :] >= ii[:, None]).astype(NP_MM)
    in_maps = []
    for b in range(B):
        xTb = np.asarray(x[b]).T.astype(NP_MM)  # [C, T]
        arr = xTb.reshape(N_C, P, N_TB, W_BLK).transpose(2, 0, 1, 3)  # [tb,c,p,t]
        xTp = np.ascontiguousarray(
            arr.reshape(N_TB, 4, 2, P, W_BLK).transpose(0, 1, 3, 2, 4))
        in_maps.append({"xTp": xTp, "consts": consts})
    return in_maps


def run(x, Wk, Wq, Wv, trace=False):
    nc = _get_nc()
    in_maps = prepare_in_maps(x, Wk, Wq, Wv)
    res = run_bass_kernel_spmd(nc, in_maps, core_ids=list(range(B)), trace=trace)
    outs = []
    for r in res.results:
        o = np.asarray(r["out"], dtype=np.float32)  # [65, T]
        outs.append((o[0:H, :] / o[H:HL, :]).T)     # normalize + transpose
    return np.stack(outs), res


def kernel(x, Wk, Wq, Wv):
    out, _ = run(x, Wk, Wq, Wv, trace=False)
    return out
